# revision 8
# baseline (speedup 1.0000x reference)
"""Trainium2 Bass kernel for nn_Attention_51539608408.

Math note: the reference applies softmax over an axis of size 1, which is
identically 1.0. Consequently the outputs depend only on `a` and `X_mask`:
    alpha[b, t] = mask[b, t] / count[b]          (count = sum_t mask)
    context[b, 0, :] = sum_t mask[b, t] * a[b, t, :] / count[b]
All the Wa/Wh/Wc/V matmuls and tanh are dead compute (verified exactly: the
reference's jax.nn.softmax subtracts the max, so exp(0)/1 == 1.0 exactly).

Sharding: pure data-parallel over batch B=32 -> 4 examples per core x 8 cores.

Per-core device kernel (per example b):
  - t axis (4096) split partition-major: t = p*32 + n  (p in [0,128), n in [0,32))
    so each SBUF partition holds a contiguous 64KB slab of DRAM -> fast DMA.
  - masked sum over t via PE matmul: lhsT = mask column [128,1], rhs = a tile
    [128, 512], accumulated over the 32 sub-tiles into PSUM [1, 512].
  - count via matmul with a ones vector, reciprocal on DVE, broadcast back to
    128 partitions with a tiny [1,128]-ones matmul.
"""

import os
import sys

import numpy as np

for _p in ("/opt/trn_rl_repo", "/opt/trn_rl_repo/concourse"):
    if os.path.isdir(_p) and _p not in sys.path:
        sys.path.insert(0, _p)

from contextlib import ExitStack

from concourse import bacc, bass, mybir, tile
from concourse.bass_utils import run_bass_kernel_spmd

B, TX, D = 32, 4096, 512
NCORES = 8
BPC = B // NCORES  # examples per core
P = 128            # SBUF partitions
NSUB = TX // P     # 32 sub-tiles along t
F32 = mybir.dt.float32
F32R = mybir.dt.float32r
U8 = mybir.dt.uint8


def build_nc(variant="f32r", chunk=8, debug=False):
    """Build the per-core Bass program.

    variant: 'f32r' (PE matmul, float32r), 'f32' (PE matmul, float32),
             'dve'  (VectorE masked accumulate + one fp32 matmul reduce)
    """
    assert NSUB % chunk == 0
    a_dt = {"f32r": F32R, "f32": F32, "dve": F32}[variant]

    nc = bacc.Bacc("TRN2", target_bir_lowering=False, debug=debug)
    a_d = nc.dram_tensor("a", [BPC, TX, D], a_dt, kind="ExternalInput")
    m_d = nc.dram_tensor("m", [BPC, TX], U8, kind="ExternalInput")
    ctx_d = nc.dram_tensor("ctx", [BPC, D], F32, kind="ExternalOutput")
    alpha_d = nc.dram_tensor("alpha", [BPC, TX], F32, kind="ExternalOutput")

    with tile.TileContext(nc) as tc, ExitStack() as ctx:
        apool = ctx.enter_context(tc.tile_pool(name="a", bufs=3))
        mpool = ctx.enter_context(tc.tile_pool(name="m", bufs=2))
        aux = ctx.enter_context(tc.tile_pool(name="aux", bufs=2))
        outp = ctx.enter_context(tc.tile_pool(name="out", bufs=2))
        const = ctx.enter_context(tc.tile_pool(name="const", bufs=1))
        pctx = ctx.enter_context(tc.tile_pool(name="pctx", bufs=2, space="PSUM"))
        pcnt = ctx.enter_context(tc.tile_pool(name="pcnt", bufs=2, space="PSUM"))

        ones_col = const.tile([P, 1], F32, tag="ones_col")
        nc.vector.memset(ones_col[:], 1.0)
        ones_row = const.tile([1, P], F32, tag="ones_row")
        nc.vector.memset(ones_row[:], 1.0)

        for b in range(BPC):
            # ---- mask: load u8, cast to a_dt ----
            m_u8 = mpool.tile([P, NSUB], U8, tag="mu8")
            nc.sync.dma_start(m_u8[:], m_d[b].rearrange("(p n) -> p n", p=P))
            m_f = mpool.tile([P, NSUB], a_dt, tag="mf")
            nc.vector.tensor_copy(m_f[:], m_u8[:])

            # ---- count and its reciprocal, broadcast to all partitions ----
            partials = aux.tile([P, 1], F32, tag="partials")
            nc.vector.reduce_sum(partials[:], m_f[:].bitcast(F32),
                                 axis=mybir.AxisListType.X)
            cnt1_ps = pcnt.tile([1, 1], F32, tag="cnt1")
            nc.tensor.matmul(cnt1_ps[:], ones_col[:], partials[:],
                             start=True, stop=True)
            cnt = aux.tile([1, 1], F32, tag="cnt_sb")
            nc.vector.tensor_copy(cnt[:], cnt1_ps[:])
            cntb_ps = pcnt.tile([P, 1], F32, tag="cntb")
            nc.tensor.matmul(cntb_ps[:], ones_row[:], cnt[:], start=True, stop=True)
            invb = aux.tile([P, 1], F32, tag="invb")
            nc.vector.reciprocal(invb[:], cntb_ps[:])

            # ---- alpha = mask * (1/count) ----
            alpha_sb = outp.tile([P, NSUB], F32, tag="alpha")
            nc.vector.tensor_scalar_mul(alpha_sb[:], m_f[:].bitcast(F32), invb[:])
            nc.sync.dma_start(alpha_d[b].rearrange("(p n) -> p n", p=P), alpha_sb[:])

            # ---- context = (1/count) * sum_t mask*a ----
            a_re = a_d[b].rearrange("(p n) d -> p n d", p=P)  # [128, 32, 512]
            ctx_ps = pctx.tile([1, D], F32, tag="ctxps")
            if variant == "dve":
                acc = apool.tile([P, D], F32, tag="acc")
                for c in range(NSUB // chunk):
                    a_t = apool.tile([P, chunk, D], a_dt, tag="achunk")
                    nc.sync.dma_start(a_t[:], a_re[:, c * chunk:(c + 1) * chunk, :])
                    for j in range(chunk):
                        n = c * chunk + j
                        if n == 0:
                            nc.vector.tensor_scalar_mul(
                                acc[:], a_t[:, j, :], m_f[:, 0:1])
                        else:
                            masked = apool.tile([P, D], F32, tag="masked")
                            nc.vector.tensor_scalar_mul(
                                masked[:], a_t[:, j, :], m_f[:, n:n + 1])
                            nc.vector.tensor_add(acc[:], acc[:], masked[:])
                nc.tensor.matmul(ctx_ps[:], ones_col[:], acc[:], start=True, stop=True)
            else:
                for c in range(NSUB // chunk):
                    a_t = apool.tile([P, chunk, D], a_dt, tag="achunk")
                    nc.sync.dma_start(a_t[:], a_re[:, c * chunk:(c + 1) * chunk, :])
                    for j in range(chunk):
                        n = c * chunk + j
                        nc.tensor.matmul(
                            ctx_ps[:],
                            m_f[:, n:n + 1],
                            a_t[:, j, :],
                            start=(n == 0),
                            stop=(n == NSUB - 1),
                        )
            ctx_sb = outp.tile([1, D], F32, tag="ctx")
            nc.vector.tensor_scalar_mul(ctx_sb[:], ctx_ps[:], invb[0:1, :])
            nc.sync.dma_start(ctx_d[b], ctx_sb[:])

    nc.compile()
    return nc


def _ensure_ntff_hook():
    """Register the axon NTFF profiling hook if the image's antenv lacks it.

    This image's ``antenv`` package has no ``axon_hooks`` module, so
    ``run_bass_kernel_spmd(trace=True)`` would skip tracing. Recreate the
    module and install the same ctypes-based hook trn_boot would have set.
    """
    import types

    try:
        import antenv.axon_hooks  # noqa: F401
        return
    except ImportError:
        pass
    try:
        import antenv
        from trn_agent_boot.trn_boot import _ntff_profile_via_ctypes
    except ImportError:
        return
    mod = types.ModuleType("antenv.axon_hooks")
    _hook = [None]
    mod.set_axon_ntff_profile_hook = lambda h: _hook.__setitem__(0, h)
    mod.get_axon_ntff_profile_hook = lambda: _hook[0]
    sys.modules["antenv.axon_hooks"] = mod
    antenv.axon_hooks = mod
    so_path = "/opt/axon/libaxon_pjrt.so"
    if os.path.exists(so_path):
        hook = _ntff_profile_via_ctypes(so_path)
        if hook is not None:
            mod.set_axon_ntff_profile_hook(hook)


_NC_CACHE = {}


def _get_nc():
    variant = os.environ.get("ATTN_KERNEL_VARIANT", "f32r")
    chunk = int(os.environ.get("ATTN_KERNEL_CHUNK", "8"))
    key = (variant, chunk)
    if key not in _NC_CACHE:
        _NC_CACHE[key] = build_nc(variant=variant, chunk=chunk, debug=False)
    return _NC_CACHE[key]


def kernel(a, h, coverage, X_mask, Wa, Wh, Wc, V, use_coverage, use_masking):
    a = np.asarray(a, dtype=np.float32)
    assert a.shape == (B, TX, D), a.shape
    masking = int(np.asarray(use_masking))
    if masking:
        m = np.asarray(X_mask).reshape(B, TX).astype(np.uint8)
    else:
        m = np.ones((B, TX), dtype=np.uint8)

    nc = _get_nc()
    in_maps = [
        {
            "a": np.ascontiguousarray(a[c * BPC:(c + 1) * BPC]),
            "m": np.ascontiguousarray(m[c * BPC:(c + 1) * BPC]),
        }
        for c in range(NCORES)
    ]
    trace = bool(int(os.environ.get("ATTN_KERNEL_TRACE", "0")))
    if trace:
        _ensure_ntff_hook()
    res = run_bass_kernel_spmd(nc, in_maps, core_ids=list(range(NCORES)),
                               trace=trace)
    if trace:
        kernel.last_exec_time_ns = res.exec_time_ns
        kernel.last_results = res
    context = np.concatenate([r["ctx"] for r in res.results], axis=0)
    alpha = np.concatenate([r["alpha"] for r in res.results], axis=0)
    if not masking:
        # softmax over the size-1 axis gives exactly 1.0 everywhere; without
        # masking there is no normalization, so undo the device-side /count.
        context = context * np.float32(TX)
        alpha = alpha * np.float32(TX)
    return context.reshape(B, 1, D), alpha


# revision 10
# speedup vs baseline: 1.1090x; 1.1090x over previous
"""Trainium2 Bass kernel for nn_Attention_51539608408.

Math note: the reference applies softmax over an axis of size 1, which is
identically 1.0. Consequently the outputs depend only on `a` and `X_mask`:
    alpha[b, t] = mask[b, t] / count[b]          (count = sum_t mask)
    context[b, 0, :] = sum_t mask[b, t] * a[b, t, :] / count[b]
All the Wa/Wh/Wc/V matmuls and tanh are dead compute (verified exactly: the
reference's jax.nn.softmax subtracts the max, so exp(0)/1 == 1.0 exactly).

Sharding: pure data-parallel over batch B=32 -> 4 examples per core x 8 cores.

Per-core device kernel (per example b):
  - t axis (4096) split partition-major: t = p*32 + n  (p in [0,128), n in [0,32))
    so each SBUF partition holds a contiguous 64KB slab of DRAM -> fast DMA.
  - masked sum over t via PE matmul: lhsT = mask column [128,1], rhs = a tile
    [128, 512], accumulated over the 32 sub-tiles into PSUM [1, 512].
  - count via matmul with a ones vector, reciprocal on DVE, broadcast back to
    128 partitions with a tiny [1,128]-ones matmul.
"""

import os
import sys

import numpy as np

for _p in ("/opt/trn_rl_repo", "/opt/trn_rl_repo/concourse"):
    if os.path.isdir(_p) and _p not in sys.path:
        sys.path.insert(0, _p)

from contextlib import ExitStack

from concourse import bacc, bass, mybir, tile
from concourse.bass_utils import run_bass_kernel_spmd

B, TX, D = 32, 4096, 512
NCORES = 8
BPC = B // NCORES  # examples per core
P = 128            # SBUF partitions
NSUB = TX // P     # 32 sub-tiles along t
F32 = mybir.dt.float32
F32R = mybir.dt.float32r
U8 = mybir.dt.uint8


def build_nc(variant="f32r", chunk=8, debug=False):
    """Build the per-core Bass program.

    variant: 'f32r' (PE matmul, float32r), 'f32' (PE matmul, float32),
             'dve'  (VectorE masked accumulate + one fp32 matmul reduce)
    """
    assert NSUB % chunk == 0
    a_dt = {"f32r": F32R, "f32": F32, "dve": F32}[variant]

    nc = bacc.Bacc("TRN2", target_bir_lowering=False, debug=debug)
    a_d = nc.dram_tensor("a", [BPC, TX, D], a_dt, kind="ExternalInput")
    m_d = nc.dram_tensor("m", [BPC, TX], U8, kind="ExternalInput")
    ctx_d = nc.dram_tensor("ctx", [BPC, D], F32, kind="ExternalOutput")
    alpha_d = nc.dram_tensor("alpha", [BPC, TX], F32, kind="ExternalOutput")

    with tile.TileContext(nc) as tc, ExitStack() as ctx:
        apool = ctx.enter_context(tc.tile_pool(name="a", bufs=5))
        mpool = ctx.enter_context(tc.tile_pool(name="m", bufs=BPC))
        aux = ctx.enter_context(tc.tile_pool(name="aux", bufs=BPC))
        outp = ctx.enter_context(tc.tile_pool(name="out", bufs=2))
        const = ctx.enter_context(tc.tile_pool(name="const", bufs=1))
        pctx = ctx.enter_context(tc.tile_pool(name="pctx", bufs=2, space="PSUM"))
        pcnt = ctx.enter_context(tc.tile_pool(name="pcnt", bufs=2, space="PSUM"))

        ones_col = const.tile([P, 1], F32, tag="ones_col")
        nc.vector.memset(ones_col[:], 1.0)
        ones_row = const.tile([1, P], F32, tag="ones_row")
        nc.vector.memset(ones_row[:], 1.0)

        # Phase 1: everything that depends only on the mask, for all examples.
        # Small DMAs ride the Scalar-engine HWDGE queue so the Sync-engine
        # queue streams the 16 big `a` chunk DMAs with no interleaved waits.
        m_fs, invbs = [], []
        for b in range(BPC):
            m_u8 = mpool.tile([P, NSUB], U8, tag="mu8")
            nc.scalar.dma_start(m_u8[:], m_d[b].rearrange("(p n) -> p n", p=P))
            m_f = mpool.tile([P, NSUB], a_dt, tag="mf")
            nc.vector.tensor_copy(m_f[:], m_u8[:])
            m_fs.append(m_f)

            # count and its reciprocal, broadcast to all partitions
            partials = aux.tile([P, 1], F32, tag="partials")
            nc.vector.reduce_sum(partials[:], m_f[:].bitcast(F32),
                                 axis=mybir.AxisListType.X)
            cnt1_ps = pcnt.tile([1, 1], F32, tag="cnt1")
            nc.tensor.matmul(cnt1_ps[:], ones_col[:], partials[:],
                             start=True, stop=True)
            cnt = aux.tile([1, 1], F32, tag="cnt_sb")
            nc.vector.tensor_copy(cnt[:], cnt1_ps[:])
            cntb_ps = pcnt.tile([P, 1], F32, tag="cntb")
            nc.tensor.matmul(cntb_ps[:], ones_row[:], cnt[:], start=True, stop=True)
            invb = aux.tile([P, 1], F32, tag="invb")
            nc.vector.reciprocal(invb[:], cntb_ps[:])
            invbs.append(invb)

            # alpha = mask * (1/count)
            alpha_sb = outp.tile([P, NSUB], F32, tag="alpha")
            nc.vector.tensor_scalar_mul(alpha_sb[:], m_f[:].bitcast(F32), invb[:])
            nc.scalar.dma_start(alpha_d[b].rearrange("(p n) -> p n", p=P),
                                alpha_sb[:])

        # Phase 2: context = (1/count) * sum_t mask*a — the memory-bound part.
        for b in range(BPC):
            m_f = m_fs[b]
            invb = invbs[b]
            a_re = a_d[b].rearrange("(p n) d -> p n d", p=P)  # [128, 32, 512]
            ctx_ps = pctx.tile([1, D], F32, tag="ctxps")
            if variant == "dve":
                acc = apool.tile([P, D], F32, tag="acc")
                for c in range(NSUB // chunk):
                    a_t = apool.tile([P, chunk, D], a_dt, tag="achunk")
                    nc.sync.dma_start(a_t[:], a_re[:, c * chunk:(c + 1) * chunk, :])
                    for j in range(chunk):
                        n = c * chunk + j
                        if n == 0:
                            nc.vector.tensor_scalar_mul(
                                acc[:], a_t[:, j, :], m_f[:, 0:1])
                        else:
                            masked = apool.tile([P, D], F32, tag="masked")
                            nc.vector.tensor_scalar_mul(
                                masked[:], a_t[:, j, :], m_f[:, n:n + 1])
                            nc.vector.tensor_add(acc[:], acc[:], masked[:])
                nc.tensor.matmul(ctx_ps[:], ones_col[:], acc[:], start=True, stop=True)
            else:
                for c in range(NSUB // chunk):
                    a_t = apool.tile([P, chunk, D], a_dt, tag="achunk")
                    nc.sync.dma_start(a_t[:], a_re[:, c * chunk:(c + 1) * chunk, :])
                    for j in range(chunk):
                        n = c * chunk + j
                        nc.tensor.matmul(
                            ctx_ps[:],
                            m_f[:, n:n + 1],
                            a_t[:, j, :],
                            start=(n == 0),
                            stop=(n == NSUB - 1),
                        )
            ctx_sb = outp.tile([1, D], F32, tag="ctx")
            nc.vector.tensor_scalar_mul(ctx_sb[:], ctx_ps[:], invb[0:1, :])
            nc.scalar.dma_start(ctx_d[b], ctx_sb[:])

    nc.compile()
    return nc


def _ensure_ntff_hook():
    """Register the axon NTFF profiling hook if the image's antenv lacks it.

    This image's ``antenv`` package has no ``axon_hooks`` module, so
    ``run_bass_kernel_spmd(trace=True)`` would skip tracing. Recreate the
    module and install the same ctypes-based hook trn_boot would have set.
    """
    import types

    try:
        import antenv.axon_hooks  # noqa: F401
        return
    except ImportError:
        pass
    try:
        import antenv
        from trn_agent_boot.trn_boot import _ntff_profile_via_ctypes
    except ImportError:
        return
    mod = types.ModuleType("antenv.axon_hooks")
    _hook = [None]
    mod.set_axon_ntff_profile_hook = lambda h: _hook.__setitem__(0, h)
    mod.get_axon_ntff_profile_hook = lambda: _hook[0]
    sys.modules["antenv.axon_hooks"] = mod
    antenv.axon_hooks = mod
    so_path = "/opt/axon/libaxon_pjrt.so"
    if os.path.exists(so_path):
        hook = _ntff_profile_via_ctypes(so_path)
        if hook is not None:
            mod.set_axon_ntff_profile_hook(hook)


_NC_CACHE = {}


def _get_nc():
    variant = os.environ.get("ATTN_KERNEL_VARIANT", "f32r")
    chunk = int(os.environ.get("ATTN_KERNEL_CHUNK", "8"))
    key = (variant, chunk)
    if key not in _NC_CACHE:
        _NC_CACHE[key] = build_nc(variant=variant, chunk=chunk, debug=False)
    return _NC_CACHE[key]


def kernel(a, h, coverage, X_mask, Wa, Wh, Wc, V, use_coverage, use_masking):
    a = np.asarray(a, dtype=np.float32)
    assert a.shape == (B, TX, D), a.shape
    masking = int(np.asarray(use_masking))
    if masking:
        m = np.asarray(X_mask).reshape(B, TX).astype(np.uint8)
    else:
        m = np.ones((B, TX), dtype=np.uint8)

    nc = _get_nc()
    in_maps = [
        {
            "a": np.ascontiguousarray(a[c * BPC:(c + 1) * BPC]),
            "m": np.ascontiguousarray(m[c * BPC:(c + 1) * BPC]),
        }
        for c in range(NCORES)
    ]
    trace = bool(int(os.environ.get("ATTN_KERNEL_TRACE", "0")))
    if trace:
        _ensure_ntff_hook()
    res = run_bass_kernel_spmd(nc, in_maps, core_ids=list(range(NCORES)),
                               trace=trace)
    if trace:
        kernel.last_exec_time_ns = res.exec_time_ns
        kernel.last_results = res
    context = np.concatenate([r["ctx"] for r in res.results], axis=0)
    alpha = np.concatenate([r["alpha"] for r in res.results], axis=0)
    if not masking:
        # softmax over the size-1 axis gives exactly 1.0 everywhere; without
        # masking there is no normalization, so undo the device-side /count.
        context = context * np.float32(TX)
        alpha = alpha * np.float32(TX)
    return context.reshape(B, 1, D), alpha
